# revision 5
# baseline (speedup 1.0000x reference)
"""Trainium2 Bass kernel for a 5-layer LSTM classifier (PaperLSTMClassifier).

Model: B=1024, T=1024, H=64, L=5 layers, V=32 vocab, variable lengths.
Strategy: data-parallel over 8 NeuronCores (128 batch columns each).

Device-side design:
  - State kept feature-major [H, B] in SBUF; cat_l = [input; h_l] is one
    [128, 128] tile so each gate matmul is out = W_packed^T.T @ cat (no
    transposes in the recurrence).
  - Re-parameterization: store Hs = h/2 and Dc = c/2. All four gates are
    computed with Sigmoid only (tanh(x) = 2*sigmoid(2x)-1 folded into
    host-scaled weights):
       i = sig(a_i), f = sig(a_f), o = sig(a_o), g' = sig(2 a_g)
       iG = (g' - 1/2) * i          (scalar_tensor_tensor)
       fD = f * Dc                  (tensor_tensor, on gpsimd)
       Dc' = iG + fD                (tensor_tensor)
       u  = sig(4 Dc')              (activation, = (tanh(c')+1)/2)
       Hs' = (u - 1/2) * o          (scalar_tensor_tensor)
    All scale factors (input x2 for layers>0 reading Hs, h-part x2, g x2,
    head x2) are folded into the weights on the host. Exact math rescaling.
  - No length masking on device: each batch column evolves independently and
    freezes after its length, so we store h4^T (transposed via PE) for every
    timestep to a DRAM ring Y and gather row (t=len[b]-1, b) at the end with
    one indirect DMA, then apply the head matmul on device.
  - Software-pipelined wavefront: wave w runs layer l at t = w - l. Static
    ramp-in/out waves; steady waves in a tc.For_i loop with U-wave unrolled
    body.
"""

import os
import numpy as np

B, T, HD, L, V = 1024, 1024, 64, 5, 32
NCORES = 8
BL = B // NCORES          # 128 batch columns per core
CAT = 2 * HD              # 128
RAMP = L - 1              # 4 ramp-in waves (and 4 ramp-out)
U = 12                    # waves per For_i iteration
NIT = (T - RAMP) // U     # 85 steady iterations (1020 steady waves)
assert RAMP + NIT * U == T

_COMPILED = {}


def _build(u, nit, ramp, tail):
    from contextlib import ExitStack

    import concourse.bass as bass
    import concourse.tile as tile
    from concourse import bacc, mybir
    from concourse.alu_op_type import AluOpType
    from concourse.bass import ds
    from concourse.masks import make_identity

    f32 = mybir.dt.float32
    i32 = mybir.dt.int32
    SIG = mybir.ActivationFunctionType.Sigmoid
    SUB = AluOpType.subtract
    MUL = AluOpType.mult
    ADD = AluOpType.add

    t_total = ramp + nit * u + tail  # == T for the real config

    nc = bacc.Bacc("TRN2", target_bir_lowering=False, debug=False)

    wmm_d = nc.dram_tensor("wmm", [L, 2, CAT, 128], f32, kind="ExternalInput")
    bias_d = nc.dram_tensor("biasv", [128, 2 * L], f32, kind="ExternalInput")
    cat0_d = nc.dram_tensor("cat0", [L, CAT, BL], f32, kind="ExternalInput")
    exs_d = nc.dram_tensor("exs", [ramp, HD, BL], f32, kind="ExternalInput")
    exm_d = nc.dram_tensor("exm", [u, nit, HD, BL], f32, kind="ExternalInput")
    gidx_d = nc.dram_tensor("gidx", [BL, 1], i32, kind="ExternalInput")
    perm_d = nc.dram_tensor("permm", [3, 128, 128], f32, kind="ExternalInput")
    headw_d = nc.dram_tensor("headw", [HD, 1], f32, kind="ExternalInput")
    out_d = nc.dram_tensor("out", [BL, 1], f32, kind="ExternalOutput")
    # Y ring: steady wave (j, i) holds t = i*u + j; tail slot [w, nit] holds
    # t = nit*u + w. Row-major rows = j*(nit+1) + i.
    y_d = nc.dram_tensor("yring", [u, nit + 1, BL, HD], f32)

    with tile.TileContext(nc) as tc, ExitStack() as ctx:
        const = ctx.enter_context(tc.tile_pool(name="const", bufs=1))
        psg = ctx.enter_context(
            tc.tile_pool(name="psg", bufs=4, space=bass.MemorySpace.PSUM)
        )
        pst = ctx.enter_context(
            tc.tile_pool(name="pst", bufs=2, space=bass.MemorySpace.PSUM)
        )
        gates = ctx.enter_context(tc.tile_pool(name="gates", bufs=4))
        prods = ctx.enter_context(tc.tile_pool(name="prods", bufs=4))
        upool = ctx.enter_context(tc.tile_pool(name="upool", bufs=4))
        ypool = ctx.enter_context(tc.tile_pool(name="ypool", bufs=4))

        # --- constants / persistent state ---
        wsb = const.tile([CAT, L, 2, 128], f32, tag="wsb")
        for l in range(L):
            for g in range(2):
                nc.sync.dma_start(wsb[:, l, g, :], wmm_d[l, g])
        bsb = const.tile([128, 2 * L], f32, tag="bsb")
        nc.sync.dma_start(bsb[:], bias_d[:])
        ident = const.tile([128, 128], f32, tag="ident")
        nc.sync.dma_start(ident[:], perm_d[0])
        perm = const.tile([128, 128], f32, tag="perm")
        nc.sync.dma_start(perm[:], perm_d[1])
        p2sb = const.tile([128, 128], f32, tag="p2sb")
        nc.sync.dma_start(p2sb[:], perm_d[2])
        gidx_sb = const.tile([BL, 1], i32, tag="gidx")
        nc.sync.dma_start(gidx_sb[:], gidx_d[:])
        headw_sb = const.tile([HD, 1], f32, tag="headw")
        nc.sync.dma_start(headw_sb[:], headw_d[:])

        cat = []
        st = []
        for l in range(L):
            c_t = const.tile([CAT, BL], f32, tag=f"cat{l}")
            nc.sync.dma_start(c_t[:], cat0_d[l])
            cat.append(c_t)
            s_t = const.tile([128, BL], f32, tag=f"st{l}")
            nc.vector.memset(s_t[64:128, :], 0.0)
            st.append(s_t)

        def layer_step(l):
            ps = psg.tile([128, 256], f32, tag="ps", bufs=4)
            nc.tensor.matmul(
                ps[:, 0:128], wsb[:, l, 0, :], cat[l][:], start=True, stop=True
            )
            nc.tensor.matmul(
                ps[:, 128:256], wsb[:, l, 1, :], cat[l][:], start=True, stop=True
            )
            sbif = gates.tile([128, BL], f32, tag="sbif")  # [i; f]
            sbgo = gates.tile([128, BL], f32, tag="sbgo")  # [g'; o]
            nc.scalar.activation(
                sbif[:], ps[:, 0:128], SIG, bias=bsb[:, 2 * l : 2 * l + 1]
            )
            nc.scalar.activation(
                sbgo[:], ps[:, 128:256], SIG, bias=bsb[:, 2 * l + 1 : 2 * l + 2]
            )
            pr = prods.tile([128, BL], f32, tag="pr")
            # lanes 0-63:  iG = (g' - 1/2) * i
            nc.vector.scalar_tensor_tensor(
                pr[0:64, :], sbgo[0:64, :], 0.5, sbif[0:64, :], SUB, MUL
            )
            # lanes 64-127: fD = f * Dc   (on gpsimd; all SBUF)
            nc.gpsimd.tensor_tensor(
                pr[64:128, :], sbif[64:128, :], st[l][64:128, :], MUL
            )
            # PE pair-sum: pd[64+j] = pr[j] + pr[64+j] = Dc'
            pd = psg.tile([128, BL], f32, tag="pd", bufs=2)
            nc.tensor.matmul(pd[:], p2sb[:], pr[:], start=True, stop=True)
            # new Dc back to state (lanes 64-127)
            nc.vector.tensor_copy(st[l][64:128, :], pd[64:128, :])
            uu = upool.tile([128, BL], f32, tag="uu")
            nc.scalar.activation(uu[64:128, :], pd[64:128, :], SIG, bias=0.0, scale=4.0)
            # Hs' = (u - 1/2) * o  -> bottom half of cat_l
            nc.vector.scalar_tensor_tensor(
                cat[l][64:128, :], uu[64:128, :], 0.5, sbgo[64:128, :], SUB, MUL
            )
            if l < L - 1:
                nc.sync.dma_start(cat[l + 1][0:64, :], cat[l][64:128, :])

        def tap(y_ap):
            # h4^T -> [BL, HD] in PSUM, then DMA to the Y ring
            pt = pst.tile([BL, 128], f32, tag="pt")
            # perm swaps halves: pt[:, 0:64] = Hs4^T, pt[:, 64:128] = input^T
            nc.tensor.transpose(pt[:], cat[L - 1][:, :], perm[:])
            ys = ypool.tile([BL, HD], f32, tag="ys")
            nc.vector.tensor_copy(ys[:], pt[:, 0:HD])
            nc.sync.dma_start(y_ap, ys[:])

        # --- ramp-in: waves 0..ramp-1, layers 0..w ---
        for w in range(ramp):
            nc.sync.dma_start(cat[0][0:64, :], exs_d[w])
            for l in range(w + 1):
                layer_step(l)

        # --- steady: waves ramp .. ramp+nit*u-1, all layers + tap ---
        with tc.For_i(
            0,
            nit,
            1,
            hint_engines=(
                mybir.EngineType.Activation,
                mybir.EngineType.DVE,
                mybir.EngineType.PE,
            ),
        ) as it:
            for j in range(u):
                nc.sync.dma_start(cat[0][0:64, :], exm_d[j, ds(it, 1), :, :])
                for l in range(L):
                    layer_step(l)
                tap(y_d[j, ds(it, 1), :, :])

        # --- ramp-out: waves T .. T+tail-1, layers w+1..L-1 ---
        for w in range(tail):
            for l in range(w + 1, L):
                layer_step(l)
            tap(y_d[w, nit, :, :])

        # --- epilogue: gather h4 at t=len-1, head matmul ---
        tc.strict_bb_all_engine_barrier()
        g4 = const.tile([BL, HD], f32, tag="g4")
        nc.gpsimd.indirect_dma_start(
            out=g4[:],
            out_offset=None,
            in_=y_d[:].rearrange("a b c d -> (a b c) d"),
            in_offset=bass.IndirectOffsetOnAxis(ap=gidx_sb[:, 0:1], axis=0),
        )
        ptr = pst.tile([HD, BL], f32, tag="pt")
        nc.tensor.transpose(ptr[:], g4[:], ident[:])
        hsb = const.tile([HD, BL], f32, tag="hsb")
        nc.scalar.copy(hsb[:], ptr[:])
        po = pst.tile([BL, 1], f32, tag="pt")
        nc.tensor.matmul(po[:], hsb[:], headw_sb[:], start=True, stop=True)
        osb = const.tile([BL, 1], f32, tag="osb")
        nc.scalar.copy(osb[:], po[:])
        nc.sync.dma_start(out_d[:], osb[:])

    nc.compile()
    return nc


def _prep_host(x, lengths, emb, W_i, W_f, W_g, W_o, b_i, b_f, b_g, b_o,
               init_h, head_w, head_b, u, nit, ramp, tail):
    """Build per-core input maps. Returns (list of in_maps, head_b scalar)."""
    x = np.asarray(x, dtype=np.int64)
    lengths = np.asarray(lengths, dtype=np.int64)
    emb = np.asarray(emb, dtype=np.float32)
    t_total = ramp + nit * u + tail

    # Folded matmul weights: lhsT[k, m]. Columns k: 0..63 input part,
    # 64..127 h part (x2 because stored state is Hs = h/2). Input part x2 for
    # layers > 0 (their input is Hs_{l-1}). g-gate rows additionally x2.
    wmm = np.empty((L, 2, CAT, 128), dtype=np.float32)
    biasv = np.empty((128, 2 * L), dtype=np.float32)
    for l in range(L):
        sx = 1.0 if l == 0 else 2.0
        col = np.concatenate(
            [np.full(HD, sx, np.float32), np.full(HD, 2.0, np.float32)]
        )  # [CAT]
        a_if = np.concatenate([W_i[l], W_f[l]], axis=0) * col  # [128, CAT]
        a_go = np.concatenate([2.0 * W_g[l], W_o[l]], axis=0) * col
        wmm[l, 0] = a_if.T.astype(np.float32)
        wmm[l, 1] = a_go.T.astype(np.float32)
        biasv[:, 2 * l] = np.concatenate([b_i[l], b_f[l]])
        biasv[:, 2 * l + 1] = np.concatenate([2.0 * b_g[l], b_o[l]])

    cat0 = np.zeros((L, CAT, BL), dtype=np.float32)
    hs0 = (np.tanh(np.asarray(init_h, dtype=np.float32)) / 2.0)  # [L, HD]
    cat0[:, HD:, :] = hs0[:, :, None]

    headw = (2.0 * np.asarray(head_w, dtype=np.float32)[0])[:, None]  # [HD,1]

    ex_all = emb[x]  # [B, T_model, H] float32

    in_maps = []
    for c in range(NCORES):
        sl = slice(c * BL, (c + 1) * BL)
        ex_c = ex_all[sl].transpose(1, 2, 0).astype(np.float32)  # [T, H, BL]
        exs = np.ascontiguousarray(ex_c[:ramp])
        exm = np.ascontiguousarray(
            ex_c[ramp : ramp + nit * u]
            .reshape(nit, u, HD, BL)
            .transpose(1, 0, 2, 3)
        )
        t_b = lengths[sl].astype(np.int64) - 1  # [BL], in [0, T-1]
        steady = t_b < nit * u
        jj = np.where(steady, t_b % u, t_b - nit * u)
        ii = np.where(steady, t_b // u, nit)
        rows = (jj * (nit + 1) + ii) * BL + np.arange(BL)
        p2m = np.zeros((128, 128), dtype=np.float32)
        for jj2 in range(64):
            p2m[jj2, 64 + jj2] = 1.0
            p2m[64 + jj2, 64 + jj2] = 1.0
        permm = np.stack(
            [np.eye(128, dtype=np.float32),
             np.roll(np.eye(128, dtype=np.float32), 64, axis=0),
             p2m]
        )
        in_maps.append(
            {
                "permm": permm,
                "wmm": wmm,
                "biasv": biasv,
                "cat0": cat0,
                "exs": exs,
                "exm": exm,
                "gidx": rows.astype(np.int32)[:, None],
                "headw": headw,
            }
        )
    return in_maps


def kernel(x, lengths, emb, W_i, W_f, W_g, W_o, b_i, b_f, b_g, b_o,
           init_h, head_w, head_b, _trace=False):
    from concourse.bass_utils import run_bass_kernel_spmd

    key = (U, NIT, RAMP, RAMP)
    if key not in _COMPILED:
        _COMPILED[key] = _build(U, NIT, RAMP, RAMP)
    nc = _COMPILED[key]

    in_maps = _prep_host(
        x, lengths, emb, W_i, W_f, W_g, W_o, b_i, b_f, b_g, b_o,
        init_h, head_w, head_b, U, NIT, RAMP, RAMP,
    )
    res = run_bass_kernel_spmd(
        nc, in_maps, list(range(NCORES)), trace=_trace
    )
    outs = [res.results[c]["out"][:, 0] for c in range(NCORES)]
    logits = np.concatenate(outs).astype(np.float32) + np.float32(
        np.asarray(head_b).reshape(-1)[0]
    )
    if _trace:
        kernel._last_exec_time_ns = res.exec_time_ns
        kernel._last_profile = res.profile_json
    return logits
